# revision 3
# baseline (speedup 1.0000x reference)
"""Trainium2 Bass kernel: 24x24-bit array multiplier (bit-vector in/out).

Inputs  A, B: [131072, 24] f32 {0,1} bits, LSB-first.
Output: [131072, 48] f32 {0,1} product bits, LSB-first.

Strategy: pure data-parallel across 8 NeuronCores (16384 rows each). Per core,
exact limb arithmetic replaces the reference's bit-level ripple adder:

  1. Horner scan on DVE (tensor_tensor_scan, data0 = {0,.5,...,.5} per 12-bit
     group) turns bits into scaled 12-bit limbs L_l * 2^-11 in one pass.
  2. Limb gathers on ScalarE (activation Copy, scale 2^11, f32->int32).
  3. 2x2 limb products: two broadcast-AP int32 tensor_tensors write all
     four partial products straight into the limb tile (the upper slots are
     re-used by the carried limbs after bd/bs consume them).
  4. Base-4096 carry propagation in int32: L = t & 4095, carry = t >> 12.
  5. Bit extraction: one broadcast-AP tensor_tensor bitwise_and against a
     (1<<s) mask table, then ScalarE Sign (int32 -> f32 {0,1}) writes the
     output bits in DRAM row-major layout directly.

Engine balance per chunk: DVE does scans + conv + carry + extract-AND;
ScalarE does gathers + Sign; constants fill DVE's initial idle window; sync DMA moves
chunks (triple-buffered). All arithmetic is exact; rel err vs reference = 0.
"""

import numpy as np

import concourse.bass as bass
import concourse.bacc as bacc
import concourse.mybir as mybir
from concourse.bass_utils import run_bass_kernel_spmd
from concourse.tile import TileContext

F32 = mybir.dt.float32
I32 = mybir.dt.int32
OP = mybir.AluOpType
AF = mybir.ActivationFunctionType

P = 128            # SBUF partitions
N_CORES = 8
N_ROWS = 131072    # total batch
N_SHARD = N_ROWS // N_CORES  # 16384 rows per core
NCHUNK = 41        # selects CHUNK_SCHEDULES[41] = [16, 64, 32, 16]


CHUNK_SCHEDULES = {
    3: [16, 48, 64],
    4: [8, 24, 32, 32, 32],
    10: [32, 80, 16],
    20: [16, 48, 56, 8],
    21: [16, 40, 48, 16, 8],
    22: [8, 40, 56, 16, 8],
    11: [16, 96, 16],
    12: [48, 64, 16],
    13: [64, 48, 16],
    14: [24, 48, 48, 8],
    15: [32, 64, 24, 8],
    16: [48, 48, 24, 8],
    17: [96, 16, 16],
    18: [64, 32, 16, 16],
    30: [32, 48, 32, 16],
    31: [24, 48, 40, 16],
    32: [24, 40, 40, 16, 8],
    33: [8, 48, 48, 16, 8],
    34: [24, 48, 32, 16, 8],
    40: [16, 48, 48, 16],
    41: [16, 64, 32, 16],
    42: [24, 64, 24, 16],
    43: [32, 64, 16, 16],
    44: [16, 32, 48, 24, 8],
    45: [32, 48, 24, 16, 8],
    46: [24, 48, 24, 16, 16],
    47: [16, 48, 32, 24, 8],
    48: [16, 32, 64, 16],
    49: [16, 32, 48, 32],
}
GATHER_ON_ACT = True
SIGN_ON_ACT = True
TAIL_ROWS = 16
TAIL_LAG = 1       # tail pieces deferred behind this many newer pieces
LAST_TAIL_ROWS = 16  # tail-piece rows for the final chunk
MID_TAIL_ROWS = 16   # tail-piece rows for early/middle chunks
OUT_DMA_ON_ACT = False
OT_BUFS = 8
WORK_BUFS = 3
IO_BUFS = 3
VT_BUFS = 4


def _build_nc(n_rows: int, nchunk: int, repeats: int = 1) -> bass.Bass:
    R = n_rows // P          # rows per partition
    if nchunk in CHUNK_SCHEDULES and R == 128:
        chunks = CHUNK_SCHEDULES[nchunk]
    else:
        chunks = [R // nchunk] * nchunk
    assert sum(chunks) == R
    Rmax = max(chunks)
    FB = Rmax * 24           # input free elems per chunk (max)
    FO = Rmax * 48           # output free elems per chunk (max)

    nc = bacc.Bacc()
    A = nc.declare_dram_parameter("A", [n_rows, 24], F32, isOutput=False)
    B = nc.declare_dram_parameter("B", [n_rows, 24], F32, isOutput=False)
    O = nc.declare_dram_parameter("out", [n_rows, 48], F32, isOutput=True)

    # partition p <-> DRAM rows [p*R, (p+1)*R); contiguous per partition
    Av = A[:].rearrange("(p r) b -> p r b", p=P)
    Bv = B[:].rearrange("(p r) b -> p r b", p=P)
    Ov = O[:].rearrange("(p r) b -> p r b", p=P)

    with TileContext(nc) as tc:
        with (
            tc.tile_pool(name="const", bufs=1) as cpool,
            tc.tile_pool(name="io", bufs=IO_BUFS) as iopool,
            tc.tile_pool(name="work", bufs=WORK_BUFS) as wpool,
            tc.tile_pool(name="vt", bufs=VT_BUFS) as vtpool,
            tc.tile_pool(name="ot", bufs=OT_BUFS) as otpool,
        ):
            # scan multiplier pattern: 0 at 12-bit group starts, 0.5
            # elsewhere. Built on the VECTOR engine: DVE is idle during the
            # first input DMA anyway, this avoids a Pool->DVE cross-engine
            # dependency before the first scan, and sidesteps any GPSIMD
            # first-call IRAM-load cost inside the measured exec.
            p02 = cpool.tile([P, FB], F32, tag="p02")
            nc.vector.memset(p02[:], 0.5)
            nc.vector.memset(
                p02[:].rearrange("p (g e) -> p g e", e=12)[:, :, 0:1], 0.0
            )
            # mask tile [P, 48] int32: mask[k*12+s] = 1 << s
            mask = cpool.tile([P, 48], I32, tag="mask")
            mask3 = mask[:].rearrange("p (k s) -> p k s", s=12)
            for s in range(12):
                nc.vector.memset(mask3[:, :, s : s + 1], 1 << s)

            pend = []
            for rep_ in range(repeats):
              r0 = 0
              for ci_, Rc in enumerate(chunks):
                carry_over = (
                    0 if (ci_ == len(chunks) - 1 and rep_ == repeats - 1)
                    else TAIL_LAG
                )
                c0 = r0
                r0 += Rc
                a_t = iopool.tile([P, Rc * 24], F32, tag="a")
                b_t = iopool.tile([P, Rc * 24], F32, tag="b")
                nc.sync.dma_start(out=a_t[:], in_=Av[:, c0 : c0 + Rc, :])
                nc.sync.dma_start(out=b_t[:], in_=Bv[:, c0 : c0 + Rc, :])

                # 1. Horner scans -> scaled limbs (L * 2^-11) at positions 12l+11
                sa_t = wpool.tile([P, Rc * 24], F32, tag="sa")
                sb_t = wpool.tile([P, Rc * 24], F32, tag="sb")
                nc.vector.tensor_tensor_scan(
                    sa_t[:], p02[:, : Rc * 24], a_t[:], 0.0, OP.mult, OP.add
                )
                nc.vector.tensor_tensor_scan(
                    sb_t[:], p02[:, : Rc * 24], b_t[:], 0.0, OP.mult, OP.add
                )
                sa = sa_t[:]
                sb = sb_t[:]

                # 2. limb gathers on ScalarE: int limbs, limb-major [l][r]
                lai = wpool.tile([P, 2 * Rc], I32, tag="lai")
                lbi = wpool.tile([P, 2 * Rc], I32, tag="lbi")
                for src_, dst in ((sa, lai), (sb, lbi)):
                    # one op per input: iterate (r, l); in strides (24, 12)
                    # offset 11, out strides (1, Rc) -> limb-major [l][r]
                    sv = src_.rearrange("p (r l e) -> p r l e", l=2, e=12)[
                        :, :, :, 11
                    ]
                    dv = dst[:].rearrange("p (l r) -> p r l", l=2)
                    if GATHER_ON_ACT:
                        nc.scalar.activation(dv, sv, AF.Copy, scale=2048.0)
                    else:
                        nc.vector.tensor_scalar(dv, sv, 2048.0, None, OP.mult)

                # 3+4. conv products and carry propagation (int32).
                # Limb-source tile lt = [t0 | u1 | u2 | L3]: the extraction
                # masks only read bits 0..11, so raw column sums go in
                # unmasked (high bits are never looked at). DVE int32 math
                # runs in fp32 internally -> keep every result < 2^24:
                # products <= 4095^2, column sums < 2^14. All exact.
                lt = wpool.tile([P, 4 * Rc], I32, tag="lt")
                pt = wpool.tile([P, 3 * Rc], I32, tag="pt")  # a0b1, a1b0, a1b1
                nc.vector.tensor_tensor(
                    lt[:, 0:Rc], lai[:, 0:Rc], lbi[:, 0:Rc], OP.mult
                )
                nc.vector.tensor_tensor(
                    pt[:, 0:Rc], lai[:, 0:Rc], lbi[:, Rc : 2 * Rc], OP.mult
                )
                nc.vector.tensor_tensor(
                    pt[:].rearrange("p (j r) -> p j r", j=3)[:, 1:3, :],
                    lai[:, Rc : 2 * Rc].unsqueeze(1).broadcast_to([P, 2, Rc]),
                    lbi[:].rearrange("p (j r) -> p j r", j=2),
                    OP.mult,
                )
                bd = wpool.tile([P, 3 * Rc], I32, tag="bd")  # lo12 of pt
                bs = wpool.tile([P, 3 * Rc], I32, tag="bs")  # hi12 of pt
                nc.vector.tensor_scalar(bd[:], pt[:], 4095, None, OP.bitwise_and)
                nc.vector.tensor_scalar(bs[:], pt[:], 12, None, OP.arith_shift_right)
                k0 = wpool.tile([P, Rc], I32, tag="k0")
                nc.vector.tensor_scalar(
                    k0[:], lt[:, 0:Rc], 12, None, OP.arith_shift_right
                )
                u1 = wpool.tile([P, Rc], I32, tag="u1")
                nc.vector.tensor_tensor(u1[:], bd[:, 0:Rc], bd[:, Rc : 2 * Rc], OP.add)
                nc.vector.tensor_tensor(lt[:, Rc : 2 * Rc], u1[:], k0[:], OP.add)
                k1a = wpool.tile([P, Rc], I32, tag="k1a")
                nc.vector.tensor_scalar(
                    k1a[:], lt[:, Rc : 2 * Rc], 12, None, OP.arith_shift_right
                )
                k1b = wpool.tile([P, Rc], I32, tag="k1b")
                nc.vector.tensor_tensor(k1b[:], k1a[:], bs[:, 0:Rc], OP.add)
                k1 = wpool.tile([P, Rc], I32, tag="k1")
                nc.vector.tensor_tensor(k1[:], k1b[:], bs[:, Rc : 2 * Rc], OP.add)
                nc.vector.tensor_tensor(
                    lt[:, 2 * Rc : 3 * Rc], bd[:, 2 * Rc :], k1[:], OP.add
                )
                k2 = wpool.tile([P, Rc], I32, tag="k2")
                nc.vector.tensor_scalar(
                    k2[:], lt[:, 2 * Rc : 3 * Rc], 12, None, OP.arith_shift_right
                )
                nc.vector.tensor_tensor(
                    lt[:, 3 * Rc :], k2[:], bs[:, 2 * Rc :], OP.add
                )

                # 5. bit extraction in tail pieces (<=TAIL_ROWS rows each):
                #    (L_k & (1<<s)) now; Sign + DMA-out deferred one chunk so
                #    ScalarE always runs the next chunk's gathers (on DVE's
                #    critical path) before the previous chunk's Sign tail.
                last_chunk = ci_ == len(chunks) - 1 and rep_ == repeats - 1
                near_end = last_chunk or ci_ == len(chunks) - 2
                tr = (
                    LAST_TAIL_ROWS if last_chunk
                    else (TAIL_ROWS if near_end else MID_TAIL_ROWS)
                )
                npc = max(1, Rc // tr)
                assert Rc % npc == 0, (Rc, npc)
                Rh = Rc // npc
                ltv = lt[:].rearrange("p (k r) -> p k r", k=4)
                for h in range(npc):
                    vt = vtpool.tile([P, Rh * 48], I32, tag=f"vt{h % 4}")
                    lt4 = (
                        ltv[:, :, h * Rh : (h + 1) * Rh]
                        .transpose([0, 2, 1])
                        .unsqueeze(3)
                        .broadcast_to([P, Rh, 4, 12])
                    )
                    mask4 = mask3.unsqueeze(1).broadcast_to([P, Rh, 4, 12])
                    nc.vector.tensor_tensor(
                        vt[:].rearrange("p (r k s) -> p r k s", k=4, s=12),
                        lt4,
                        mask4,
                        OP.bitwise_and,
                    )
                    pend.append((vt, c0 + h * Rh, Rh))
                while len(pend) > carry_over:
                    vt, row0, Rh_ = pend.pop(0)
                    o_t = otpool.tile([P, Rh_ * 48], F32, tag="o")
                    # very last piece: compare on DVE to skip the DVE->ACT hop
                    last_piece = carry_over == 0 and not pend
                    if SIGN_ON_ACT and not last_piece:
                        nc.scalar.activation(o_t[:], vt[:], AF.Sign)
                    else:
                        nc.vector.tensor_scalar(o_t[:], vt[:], 0, None, OP.is_gt)
                    if OUT_DMA_ON_ACT:
                        nc.scalar.dma_start(
                            out=Ov[:, row0 : row0 + Rh_, :], in_=o_t[:]
                        )
                    else:
                        nc.sync.dma_start(
                            out=Ov[:, row0 : row0 + Rh_, :], in_=o_t[:]
                        )

    nc.finalize()
    return nc


_CACHE = {}


def _get_nc():
    key = (N_SHARD, NCHUNK)
    if key not in _CACHE:
        _CACHE[key] = _build_nc(N_SHARD, NCHUNK)
    return _CACHE[key]


def kernel(A: np.ndarray, B: np.ndarray) -> np.ndarray:
    A = np.ascontiguousarray(A, dtype=np.float32)
    B = np.ascontiguousarray(B, dtype=np.float32)
    nc = _get_nc()
    in_maps = [
        {
            "A": A[c * N_SHARD : (c + 1) * N_SHARD],
            "B": B[c * N_SHARD : (c + 1) * N_SHARD],
        }
        for c in range(N_CORES)
    ]
    res = run_bass_kernel_spmd(nc, in_maps, core_ids=list(range(N_CORES)))
    return np.concatenate([res.results[i]["out"] for i in range(N_CORES)], axis=0)



# revision 4
# speedup vs baseline: 1.0008x; 1.0008x over previous
"""Trainium2 Bass kernel: 24x24-bit array multiplier (bit-vector in/out).

Inputs  A, B: [131072, 24] f32 {0,1} bits, LSB-first.
Output: [131072, 48] f32 {0,1} product bits, LSB-first.

Strategy: pure data-parallel across 8 NeuronCores (16384 rows each). Per core,
exact limb arithmetic replaces the reference's bit-level ripple adder:

  1. Horner scan on DVE (tensor_tensor_scan, data0 = {0,.5,...,.5} per 12-bit
     group) turns bits into scaled 12-bit limbs L_l * 2^-11 in one pass.
  2. Limb gathers on ScalarE (activation Copy, scale 2^11, f32->int32).
  3. 2x2 limb products: two broadcast-AP int32 tensor_tensors write all
     four partial products straight into the limb tile (the upper slots are
     re-used by the carried limbs after bd/bs consume them).
  4. Base-4096 carry propagation in int32: L = t & 4095, carry = t >> 12.
  5. Bit extraction: one broadcast-AP tensor_tensor bitwise_and against a
     (1<<s) mask table, then ScalarE Sign (int32 -> f32 {0,1}) writes the
     output bits in DRAM row-major layout directly.

Engine balance per chunk: DVE does scans + conv + carry + extract-AND;
ScalarE does gathers + Sign; constants fill DVE's initial idle window; sync DMA moves
chunks (triple-buffered). All arithmetic is exact; rel err vs reference = 0.
"""

import numpy as np

import concourse.bass as bass
import concourse.bacc as bacc
import concourse.mybir as mybir
from concourse.bass_utils import run_bass_kernel_spmd
from concourse.tile import TileContext

F32 = mybir.dt.float32
I32 = mybir.dt.int32
OP = mybir.AluOpType
AF = mybir.ActivationFunctionType

P = 128            # SBUF partitions
N_CORES = 8
N_ROWS = 131072    # total batch
N_SHARD = N_ROWS // N_CORES  # 16384 rows per core
NCHUNK = 41        # selects CHUNK_SCHEDULES[41] = [16, 64, 32, 16]


CHUNK_SCHEDULES = {
    3: [16, 48, 64],
    4: [8, 24, 32, 32, 32],
    10: [32, 80, 16],
    20: [16, 48, 56, 8],
    21: [16, 40, 48, 16, 8],
    22: [8, 40, 56, 16, 8],
    11: [16, 96, 16],
    12: [48, 64, 16],
    13: [64, 48, 16],
    14: [24, 48, 48, 8],
    15: [32, 64, 24, 8],
    16: [48, 48, 24, 8],
    17: [96, 16, 16],
    18: [64, 32, 16, 16],
    30: [32, 48, 32, 16],
    31: [24, 48, 40, 16],
    32: [24, 40, 40, 16, 8],
    33: [8, 48, 48, 16, 8],
    34: [24, 48, 32, 16, 8],
    40: [16, 48, 48, 16],
    41: [16, 64, 32, 16],
    42: [24, 64, 24, 16],
    43: [32, 64, 16, 16],
    44: [16, 32, 48, 24, 8],
    45: [32, 48, 24, 16, 8],
    46: [24, 48, 24, 16, 16],
    47: [16, 48, 32, 24, 8],
    48: [16, 32, 64, 16],
    49: [16, 32, 48, 32],
}
GATHER_ON_ACT = True
SIGN_ON_ACT = True
TAIL_ROWS = 16
TAIL_LAG = 1       # tail pieces deferred behind this many newer pieces
LAST_TAIL_ROWS = 16  # tail-piece rows for the final chunk
MID_TAIL_ROWS = 16   # tail-piece rows for early/middle chunks
OUT_DMA_ON_ACT = False
OT_BUFS = 8
WORK_BUFS = 3
IO_BUFS = 3
VT_BUFS = 4


def _build_nc(n_rows: int, nchunk: int, repeats: int = 1) -> bass.Bass:
    R = n_rows // P          # rows per partition
    if nchunk in CHUNK_SCHEDULES and R == 128:
        chunks = CHUNK_SCHEDULES[nchunk]
    else:
        chunks = [R // nchunk] * nchunk
    assert sum(chunks) == R
    Rmax = max(chunks)
    FB = Rmax * 24           # input free elems per chunk (max)
    FO = Rmax * 48           # output free elems per chunk (max)

    nc = bacc.Bacc()
    A = nc.declare_dram_parameter("A", [n_rows, 24], F32, isOutput=False)
    B = nc.declare_dram_parameter("B", [n_rows, 24], F32, isOutput=False)
    O = nc.declare_dram_parameter("out", [n_rows, 48], F32, isOutput=True)

    # partition p <-> DRAM rows [p*R, (p+1)*R); contiguous per partition
    Av = A[:].rearrange("(p r) b -> p r b", p=P)
    Bv = B[:].rearrange("(p r) b -> p r b", p=P)
    Ov = O[:].rearrange("(p r) b -> p r b", p=P)

    with TileContext(nc) as tc:
        with (
            tc.tile_pool(name="const", bufs=1) as cpool,
            tc.tile_pool(name="io", bufs=IO_BUFS) as iopool,
            tc.tile_pool(name="work", bufs=WORK_BUFS) as wpool,
            tc.tile_pool(name="vt", bufs=VT_BUFS) as vtpool,
            tc.tile_pool(name="ot", bufs=OT_BUFS) as otpool,
        ):
            # scan multiplier pattern: 0 at 12-bit group starts, 0.5
            # elsewhere. Built on the VECTOR engine: DVE is idle during the
            # first input DMA anyway, this avoids a Pool->DVE cross-engine
            # dependency before the first scan, and sidesteps any GPSIMD
            # first-call IRAM-load cost inside the measured exec.
            p02 = cpool.tile([P, FB], F32, tag="p02")
            nc.vector.memset(p02[:], 0.5)
            nc.vector.memset(
                p02[:].rearrange("p (g e) -> p g e", e=12)[:, :, 0:1], 0.0
            )
            # mask tile [P, 48] int32: mask[k*12+s] = 1 << s
            mask = cpool.tile([P, 48], I32, tag="mask")
            mask3 = mask[:].rearrange("p (k s) -> p k s", s=12)
            for s in range(12):
                nc.vector.memset(mask3[:, :, s : s + 1], 1 << s)

            pend = []
            for rep_ in range(repeats):
              r0 = 0
              for ci_, Rc in enumerate(chunks):
                carry_over = (
                    0 if (ci_ == len(chunks) - 1 and rep_ == repeats - 1)
                    else TAIL_LAG
                )
                c0 = r0
                r0 += Rc
                a_t = iopool.tile([P, Rc * 24], F32, tag="a")
                b_t = iopool.tile([P, Rc * 24], F32, tag="b")
                nc.sync.dma_start(out=a_t[:], in_=Av[:, c0 : c0 + Rc, :])
                nc.sync.dma_start(out=b_t[:], in_=Bv[:, c0 : c0 + Rc, :])

                # 1. Horner scans -> scaled limbs (L * 2^-11) at positions 12l+11
                sa_t = wpool.tile([P, Rc * 24], F32, tag="sa")
                sb_t = wpool.tile([P, Rc * 24], F32, tag="sb")
                nc.vector.tensor_tensor_scan(
                    sa_t[:], p02[:, : Rc * 24], a_t[:], 0.0, OP.mult, OP.add
                )
                nc.vector.tensor_tensor_scan(
                    sb_t[:], p02[:, : Rc * 24], b_t[:], 0.0, OP.mult, OP.add
                )
                sa = sa_t[:]
                sb = sb_t[:]

                # 2. limb gathers on ScalarE: int limbs, limb-major [l][r]
                lai = wpool.tile([P, 2 * Rc], I32, tag="lai")
                lbi = wpool.tile([P, 2 * Rc], I32, tag="lbi")
                for src_, dst in ((sa, lai), (sb, lbi)):
                    # one op per input: iterate (r, l); in strides (24, 12)
                    # offset 11, out strides (1, Rc) -> limb-major [l][r]
                    sv = src_.rearrange("p (r l e) -> p r l e", l=2, e=12)[
                        :, :, :, 11
                    ]
                    dv = dst[:].rearrange("p (l r) -> p r l", l=2)
                    if GATHER_ON_ACT:
                        nc.scalar.activation(dv, sv, AF.Copy, scale=2048.0)
                    else:
                        nc.vector.tensor_scalar(dv, sv, 2048.0, None, OP.mult)

                # 3+4. conv products and carry propagation (int32).
                # Limb-source tile lt = [t0 | u1 | u2 | L3]: the extraction
                # masks only read bits 0..11, so raw column sums go in
                # unmasked (high bits are never looked at). DVE int32 math
                # runs in fp32 internally -> keep every result < 2^24:
                # products <= 4095^2, column sums < 2^14. All exact.
                lt = wpool.tile([P, 4 * Rc], I32, tag="lt")
                pt = wpool.tile([P, 3 * Rc], I32, tag="pt")  # a0b1, a1b0, a1b1
                nc.vector.tensor_tensor(
                    lt[:, 0:Rc], lai[:, 0:Rc], lbi[:, 0:Rc], OP.mult
                )
                nc.vector.tensor_tensor(
                    pt[:, 0:Rc], lai[:, 0:Rc], lbi[:, Rc : 2 * Rc], OP.mult
                )
                nc.vector.tensor_tensor(
                    pt[:].rearrange("p (j r) -> p j r", j=3)[:, 1:3, :],
                    lai[:, Rc : 2 * Rc].unsqueeze(1).broadcast_to([P, 2, Rc]),
                    lbi[:].rearrange("p (j r) -> p j r", j=2),
                    OP.mult,
                )
                bd = wpool.tile([P, 3 * Rc], I32, tag="bd")  # lo12 of pt
                bs = wpool.tile([P, 3 * Rc], I32, tag="bs")  # hi12 of pt
                nc.vector.tensor_scalar(bd[:], pt[:], 4095, None, OP.bitwise_and)
                nc.vector.tensor_scalar(bs[:], pt[:], 12, None, OP.arith_shift_right)
                k0 = wpool.tile([P, Rc], I32, tag="k0")
                nc.vector.tensor_scalar(
                    k0[:], lt[:, 0:Rc], 12, None, OP.arith_shift_right
                )
                u1 = wpool.tile([P, Rc], I32, tag="u1")
                nc.vector.tensor_tensor(u1[:], bd[:, 0:Rc], bd[:, Rc : 2 * Rc], OP.add)
                nc.vector.tensor_tensor(lt[:, Rc : 2 * Rc], u1[:], k0[:], OP.add)
                # s01 = bs0+bs1 runs off the critical path, cutting the
                # serial carry chain from 9 to 7 dependency links
                s01 = wpool.tile([P, Rc], I32, tag="s01")
                nc.vector.tensor_tensor(
                    s01[:], bs[:, 0:Rc], bs[:, Rc : 2 * Rc], OP.add
                )
                k1a = wpool.tile([P, Rc], I32, tag="k1a")
                nc.vector.tensor_scalar(
                    k1a[:], lt[:, Rc : 2 * Rc], 12, None, OP.arith_shift_right
                )
                k1 = wpool.tile([P, Rc], I32, tag="k1")
                nc.vector.tensor_tensor(k1[:], k1a[:], s01[:], OP.add)
                nc.vector.tensor_tensor(
                    lt[:, 2 * Rc : 3 * Rc], bd[:, 2 * Rc :], k1[:], OP.add
                )
                k2 = wpool.tile([P, Rc], I32, tag="k2")
                nc.vector.tensor_scalar(
                    k2[:], lt[:, 2 * Rc : 3 * Rc], 12, None, OP.arith_shift_right
                )
                nc.vector.tensor_tensor(
                    lt[:, 3 * Rc :], k2[:], bs[:, 2 * Rc :], OP.add
                )

                # 5. bit extraction in tail pieces (<=TAIL_ROWS rows each):
                #    (L_k & (1<<s)) now; Sign + DMA-out deferred one chunk so
                #    ScalarE always runs the next chunk's gathers (on DVE's
                #    critical path) before the previous chunk's Sign tail.
                last_chunk = ci_ == len(chunks) - 1 and rep_ == repeats - 1
                near_end = last_chunk or ci_ == len(chunks) - 2
                tr = (
                    LAST_TAIL_ROWS if last_chunk
                    else (TAIL_ROWS if near_end else MID_TAIL_ROWS)
                )
                npc = max(1, Rc // tr)
                assert Rc % npc == 0, (Rc, npc)
                Rh = Rc // npc
                ltv = lt[:].rearrange("p (k r) -> p k r", k=4)
                for h in range(npc):
                    vt = vtpool.tile([P, Rh * 48], I32, tag=f"vt{h % 4}")
                    lt4 = (
                        ltv[:, :, h * Rh : (h + 1) * Rh]
                        .transpose([0, 2, 1])
                        .unsqueeze(3)
                        .broadcast_to([P, Rh, 4, 12])
                    )
                    mask4 = mask3.unsqueeze(1).broadcast_to([P, Rh, 4, 12])
                    nc.vector.tensor_tensor(
                        vt[:].rearrange("p (r k s) -> p r k s", k=4, s=12),
                        lt4,
                        mask4,
                        OP.bitwise_and,
                    )
                    pend.append((vt, c0 + h * Rh, Rh))
                while len(pend) > carry_over:
                    vt, row0, Rh_ = pend.pop(0)
                    o_t = otpool.tile([P, Rh_ * 48], F32, tag="o")
                    # very last piece: compare on DVE to skip the DVE->ACT hop
                    last_piece = carry_over == 0 and not pend
                    if SIGN_ON_ACT and not last_piece:
                        nc.scalar.activation(o_t[:], vt[:], AF.Sign)
                    else:
                        nc.vector.tensor_scalar(o_t[:], vt[:], 0, None, OP.is_gt)
                    if OUT_DMA_ON_ACT:
                        nc.scalar.dma_start(
                            out=Ov[:, row0 : row0 + Rh_, :], in_=o_t[:]
                        )
                    else:
                        nc.sync.dma_start(
                            out=Ov[:, row0 : row0 + Rh_, :], in_=o_t[:]
                        )

    nc.finalize()
    return nc


_CACHE = {}


def _get_nc():
    key = (N_SHARD, NCHUNK)
    if key not in _CACHE:
        _CACHE[key] = _build_nc(N_SHARD, NCHUNK)
    return _CACHE[key]


def kernel(A: np.ndarray, B: np.ndarray) -> np.ndarray:
    A = np.ascontiguousarray(A, dtype=np.float32)
    B = np.ascontiguousarray(B, dtype=np.float32)
    nc = _get_nc()
    in_maps = [
        {
            "A": A[c * N_SHARD : (c + 1) * N_SHARD],
            "B": B[c * N_SHARD : (c + 1) * N_SHARD],
        }
        for c in range(N_CORES)
    ]
    res = run_bass_kernel_spmd(nc, in_maps, core_ids=list(range(N_CORES)))
    return np.concatenate([res.results[i]["out"] for i in range(N_CORES)], axis=0)

